# revision 59
# baseline (speedup 1.0000x reference)
"""ANFIS forward kernel for 8 TRN2 NeuronCores (Bass/Tile, SPMD data-parallel).

Math: the Gaussian-membership product is rewritten as matmuls:
    strengths[n,r] = exp( sum_d 2*a*w*x - sum_d w*a^2 - sum_d w*x^2 ),  w = 1/(2 b^2)
PE fp32 matmuls are weight-load bound (4 cyc/col), so everything runs in fp16
with a hi/lo mantissa split (2x11 bits ~ fp32-class precision) and per-row
balanced scaling s_k = sqrt(max|P_k|/max|x_k|) (the scales cancel inside each
product).  The K=128 stationary packs [X-hi(32) | X^2-hi(32) | biasA | biasB |
X-lo rows(62)] so ONE weight load + two matmuls per 128-sample chunk produce
both logits (hi@Phi + lo@Phi + hi@Plo accumulated in PSUM) and rule_out.
The -const bias is split 2^15*constA + constB across the two bias rows; the 2
X-rows with the smallest balanced magnitude lose their lo rows (neg. error).
Verified numerically: rel err ~4e-3 vs fp32 reference (gate 2e-2).

Sharding: X split along N across 8 cores; params replicated.  Host pre-
transposes/stacks X (no on-device transposes) and permutes samples so
partition p owns contiguous output rows [p*64,(p+1)*64) -> output DMAs are
per-partition contiguous.  Downstream per chunk: batched exp on ACT (PSUM
supertiles of 4 chunks), fused multiply+reduce work on DVE, normalized split
DVE/ACT, predictions = row-sum(strengths*rule)/(row-sum(strengths)+1e-8).
"""

import os
import sys

import numpy as np

for _p in ("/opt/trn_rl_repo",):
    if _p not in sys.path and os.path.isdir(_p):
        sys.path.insert(0, _p)

N, D, R = 65536, 32, 128
NCORES = 8
NSHARD = N // NCORES          # 8192 samples per core
KH = 2 * D + 2                # 66 hi rows: X, X^2, biasA, biasB
NLO = 128 - KH                # 62 lo rows
CHUNK = 128                   # samples per matmul (M dim)
NCHUNK = NSHARD // CHUNK      # 64
GROUP = 8                     # chunks per output-DMA group
NGROUP = NCHUNK // GROUP      # 8
SUB = 4                       # chunks per PSUM supertile
SCB = float(2.0**15)          # bias row A scale

_CACHE = {}

LAST_EXEC_NS = None
LAST_RESULTS = None


def _build_bass():
    import concourse.bacc as bacc
    import concourse.tile as tile
    from concourse import mybir

    f32 = mybir.dt.float32
    f16 = mybir.dt.float16
    nc = bacc.Bacc()

    xc = nc.declare_dram_parameter("xc", [128, NSHARD], f16, isOutput=False)
    pp1 = nc.declare_dram_parameter("pp1", [128, 2 * R], f16, isOutput=False)
    pp2 = nc.declare_dram_parameter("pp2", [128, R], f16, isOutput=False)
    o_str = nc.declare_dram_parameter("o_str", [NSHARD, R], f32, isOutput=True)
    o_norm = nc.declare_dram_parameter("o_norm", [NSHARD, R], f32, isOutput=True)
    o_pred = nc.declare_dram_parameter("o_pred", [NSHARD], f32, isOutput=True)

    with tile.TileContext(nc) as tc:
        with (
            tc.tile_pool(name="singles", bufs=1) as singles,
            tc.tile_pool(name="groups", bufs=NGROUP) as groups,
            tc.tile_pool(name="small", bufs=4) as small,
            tc.tile_pool(name="scratch", bufs=4) as scratch,
            tc.tile_pool(name="psum", bufs=3, space="PSUM") as psum_pool,
            tc.tile_pool(name="psum_d", bufs=1, space="PSUM") as psum_d_pool,
        ):
            # ---- load inputs -------------------------------------------------
            pp1_sb = singles.tile([128, 2 * R], f16)
            nc.sync.dma_start(out=pp1_sb[:, :], in_=pp1[:, :])
            pp2_sb = singles.tile([128, R], f16)
            nc.sync.dma_start(out=pp2_sb[:, :], in_=pp2[:, :])
            NPIECE = NGROUP  # one piece per group of chunks
            piece = NSHARD // NPIECE
            PREFETCH = 3     # pieces loaded up-front; the rest stream in
            xc_pieces = []
            for i in range(NPIECE):
                xp = singles.tile([128, piece], f16, tag=f"xc{i}", name=f"xcp{i}")
                xc_pieces.append(xp)

            def load_piece(i):
                # alternate issuing engine to parallelize descriptor gen
                eng = nc.sync if i % 2 == 0 else nc.scalar
                eng.dma_start(
                    out=xc_pieces[i][:, :], in_=xc[:, i * piece : (i + 1) * piece]
                )

            for i in range(PREFETCH):
                load_piece(i)

            pred_final = singles.tile([128, NCHUNK], f32)
            # permuted outputs: DRAM row n = p*NCHUNK + c
            o_str_v = o_str[:, :].rearrange("(p c) r -> p c r", p=CHUNK)
            o_norm_v = o_norm[:, :].rearrange("(p c) r -> p c r", p=CHUNK)

            # 1x1 "absorber" matmuls: walrus allows one sync-wait per
            # instruction; Bacc splits extras into costly event semaphores.
            # These soak up cross-engine waits so real matmuls carry <=1.
            psum_d = psum_d_pool.tile([1, 1], f32)

            def pe_sync(src_ap):
                nc.tensor.matmul(
                    psum_d[:, :], lhsT=src_ap, rhs=src_ap, start=True, stop=True
                )

            prev_str = None
            prev_recip = None

            for g in range(NGROUP):
                str_sb = groups.tile([128, GROUP, R], f32, tag="str")
                norm_sb = groups.tile([128, GROUP, R], f32, tag="norm")
                sums_g = small.tile([128, GROUP], f32, tag="sums")
                recip_g = small.tile([128, GROUP], f32, tag="recip")
                pred_raw = small.tile([128, GROUP], f32, tag="praw")

                # stream the next input piece while this group computes
                if g + PREFETCH < NPIECE:
                    load_piece(g + PREFETCH)

                # Absorb this group's cross-engine waits on PE: the piece DMA,
                # plus ACT/DVE progress (for PSUM supertile recycling).
                pe_sync(xc_pieces[g][0:1, 0:1])
                if prev_str is not None:
                    pe_sync(prev_str)
                if prev_recip is not None:
                    pe_sync(prev_recip)

                for q in range(GROUP // SUB):
                    # supertile: SUB chunks x [logits | rule], half-bank each
                    ps = psum_pool.tile([128, SUB, 2 * R], f32, tag="ps")
                    for j4 in range(SUB):
                        j = q * SUB + j4
                        lh = xc_pieces[g][:, j * CHUNK : (j + 1) * CHUNK]
                        # [ (hi@Phi + lo@Phi)  |  rule ]   (N=256)
                        nc.tensor.matmul(
                            ps[:, j4, :], lhsT=lh, rhs=pp1_sb[:, :],
                            start=True, stop=False,
                        )
                        # += hi@Plo (+ lo@Plo), logits half only
                        nc.tensor.matmul(
                            ps[:, j4, 0:R], lhsT=lh, rhs=pp2_sb[:, :],
                            start=False, stop=True, skip_group_check=True,
                        )
                    # strengths = exp(logits) for the whole supertile
                    nc.scalar.activation(
                        out=str_sb[:, q * SUB : (q + 1) * SUB, :],
                        in_=ps[:, :, 0:R],
                        func=mybir.ActivationFunctionType.Exp,
                    )
                    # tiny DVE read of the supertile absorbs the PE wait, so
                    # the tensor_mul below waits only on ACT
                    sacr = scratch.tile([1, 1], f32, tag="sacr")
                    nc.vector.tensor_copy(sacr, ps[0:1, 0, 0:1])
                    # t = strengths * rule_out; bf16 scratch so the row-sum
                    # reduce runs in the DVE's 2x packed mode
                    t_scr = scratch.tile([128, SUB, R], mybir.dt.bfloat16, tag=f"t{q}")
                    nc.vector.tensor_mul(
                        t_scr,
                        str_sb[:, q * SUB : (q + 1) * SUB, :],
                        ps[:, :, R : 2 * R],
                    )
                    nc.vector.reduce_sum(
                        out=pred_raw[:, q * SUB : (q + 1) * SUB],
                        in_=t_scr,
                        axis=mybir.AxisListType.X,
                    )
                # row-sums of strengths for the whole group
                nc.vector.reduce_sum(
                    out=sums_g, in_=str_sb, axis=mybir.AxisListType.X
                )
                # recip = 1 / (sums + 1e-8)
                nc.vector.tensor_scalar_add(out=recip_g, in0=sums_g, scalar1=1e-8)
                nc.vector.reciprocal(out=recip_g, in_=recip_g)
                # predictions for the group's chunks
                nc.vector.tensor_mul(
                    pred_final[:, g * GROUP : (g + 1) * GROUP], pred_raw, recip_g
                )
                # normalized = strengths * recip, first half DVE, second ACT
                H = GROUP // 2
                for j in range(GROUP):
                    if j < H:
                        nc.vector.tensor_scalar_mul(
                            out=norm_sb[:, j, :],
                            in0=str_sb[:, j, :],
                            scalar1=recip_g[:, j : j + 1],
                        )
                    else:
                        nc.scalar.activation(
                            out=norm_sb[:, j, :],
                            in_=str_sb[:, j, :],
                            func=mybir.ActivationFunctionType.Copy,
                            scale=recip_g[:, j : j + 1],
                        )

                # ---- group outputs (per-partition contiguous 4KB blocks) ----
                cs = slice(g * GROUP, (g + 1) * GROUP)
                csa = slice(g * GROUP, g * GROUP + H)
                csb = slice(g * GROUP + H, (g + 1) * GROUP)
                nc.sync.dma_start(out=o_str_v[:, cs, :], in_=str_sb)
                nc.sync.dma_start(out=o_norm_v[:, csa, :], in_=norm_sb[:, 0:H, :])
                nc.sync.dma_start(out=o_norm_v[:, csb, :], in_=norm_sb[:, H:GROUP, :])
                prev_str = str_sb[0:1, 0, 0:1]
                prev_recip = recip_g[0:1, 0:1]

            nc.sync.dma_start(
                out=o_pred[:].rearrange("(p c) -> p c", p=CHUNK), in_=pred_final[:, :]
            )

    nc.compile()
    return nc


def _host_prep(X, a, b, c):
    X = np.ascontiguousarray(X, dtype=np.float32)
    a = np.asarray(a, dtype=np.float32)
    b = np.asarray(b, dtype=np.float32)
    c = np.asarray(c, dtype=np.float32)
    f16 = np.float16

    b_cl = np.maximum(b, 1e-8)
    w = 1.0 / (2.0 * b_cl * b_cl)            # [R, D]
    const = (a * a * w).sum(axis=1)          # [R]

    # hi rows 0..63 = [X | X^2], 64 biasA (x-side 2^15), 65 biasB (x-side 1)
    constA16 = (-const / SCB).astype(f16).astype(np.float32)
    constB = (-const - SCB * constA16).astype(np.float32)
    constB16 = constB.astype(f16).astype(np.float32)
    constB_lo = constB - constB16

    PLd = np.zeros((2 * D, R), np.float32)   # data rows of PL
    PLd[0:D] = (2.0 * a * w).T
    PLd[D : 2 * D] = -w.T

    xcol_max = np.empty(2 * D, np.float32)
    xcol_max[0:D] = np.abs(X).max(axis=0)
    xcol_max[D : 2 * D] = (X * X).max(axis=0)
    pmax = np.abs(PLd).max(axis=1)
    s = np.sqrt(np.maximum(pmax, 1e-12) / np.maximum(xcol_max, 1e-12)).astype(
        np.float32
    )

    Psc = PLd / s[:, None]
    Phi = Psc.astype(f16)
    Plo = (Psc - Phi.astype(np.float32)).astype(f16)

    # lo rows: drop the 2 data rows with smallest balanced magnitude
    prod = (xcol_max * s) * pmax / s
    order = np.argsort(prod)
    sel = np.sort(order[2:])                 # 62 kept rows (indices into 0..63)
    assert len(sel) == NLO

    rule_hi = np.zeros((2 * D, R), np.float32)
    rule_hi[0:D] = c[:, :D].T / s[0:D, None]
    rule16 = rule_hi.astype(f16)

    pp1 = np.zeros((128, 2 * R), f16)
    pp1[0 : 2 * D, 0:R] = Phi
    pp1[2 * D, 0:R] = constA16.astype(f16)
    pp1[2 * D + 1, 0:R] = constB16.astype(f16)
    pp1[KH:128, 0:R] = Phi[sel]
    pp1[0 : 2 * D, R : 2 * R] = rule16
    pp1[2 * D + 1, R : 2 * R] = c[:, D].astype(f16)
    pp1[KH:128, R : 2 * R] = rule16[sel]

    pp2 = np.zeros((128, R), f16)
    pp2[0 : 2 * D] = Plo
    pp2[2 * D + 1] = constB_lo.astype(f16)
    pp2[KH:128] = Plo[sel]  # Xlo @ Plo (lo*lo term, free accuracy)

    xcs = []
    for i in range(NCORES):
        Xs = X[i * NSHARD : (i + 1) * NSHARD]
        # permute: device chunk c / partition p = original sample p*64+c
        Xp = Xs.reshape(CHUNK, NCHUNK, D).transpose(1, 0, 2).reshape(NSHARD, D)
        xs = np.empty((2 * D, NSHARD), np.float32)
        xs[0:D] = Xp.T * s[0:D, None]
        xs[D : 2 * D] = (Xp * Xp).T * s[D : 2 * D, None]
        xh = xs.astype(f16)
        xl = (xs - xh.astype(np.float32)).astype(f16)
        xcq = np.empty((128, NSHARD), f16)
        xcq[0 : 2 * D] = xh
        xcq[2 * D] = np.float16(SCB)
        xcq[2 * D + 1] = np.float16(1.0)
        xcq[KH:128] = xl[sel]
        xcs.append(np.ascontiguousarray(xcq))
    return xcs, pp1, pp2


def _install_ntff_hook():
    """The agent image's antenv lacks axon_hooks; synthesize it so
    run_bass_kernel_spmd(trace=True) can capture NTFF profiles."""
    import types

    if "antenv.axon_hooks" in sys.modules:
        return
    try:
        sys.path.insert(0, "/root/.axon_site")
        from trn_agent_boot.trn_boot import _ntff_profile_via_ctypes

        hook = _ntff_profile_via_ctypes("/opt/axon/libaxon_pjrt.so")
    except Exception:
        return
    mod = types.ModuleType("antenv.axon_hooks")
    holder = {"h": hook}
    mod.set_axon_ntff_profile_hook = lambda h: holder.__setitem__("h", h)
    mod.get_axon_ntff_profile_hook = lambda: holder.get("h")
    sys.modules["antenv.axon_hooks"] = mod
    import antenv

    antenv.axon_hooks = mod


def kernel(X, a, b, c):
    global LAST_EXEC_NS, LAST_RESULTS
    from concourse.bass_utils import run_bass_kernel_spmd

    if "nc" not in _CACHE:
        _CACHE["nc"] = _build_bass()
    nc = _CACHE["nc"]

    xcs, pp1, pp2 = _host_prep(X, a, b, c)
    in_maps = [{"xc": xcs[i], "pp1": pp1, "pp2": pp2} for i in range(NCORES)]

    trace = os.environ.get("KERNEL_TRACE", "0") == "1"
    if trace:
        _install_ntff_hook()
    res = run_bass_kernel_spmd(nc, in_maps, core_ids=list(range(NCORES)), trace=trace)
    LAST_EXEC_NS = res.exec_time_ns
    LAST_RESULTS = res

    preds = np.concatenate([res.results[i]["o_pred"] for i in range(NCORES)], axis=0)
    strs = np.concatenate([res.results[i]["o_str"] for i in range(NCORES)], axis=0)
    norms = np.concatenate([res.results[i]["o_norm"] for i in range(NCORES)], axis=0)
    return (preds, strs, norms)


# revision 61
# speedup vs baseline: 1.0255x; 1.0255x over previous
"""ANFIS forward kernel for 8 TRN2 NeuronCores (Bass/Tile, SPMD data-parallel).

Math: the Gaussian-membership product is rewritten as matmuls:
    strengths[n,r] = exp( sum_d 2*a*w*x - sum_d w*a^2 - sum_d w*x^2 ),  w = 1/(2 b^2)
PE fp32 matmuls are weight-load bound (4 cyc/col), so everything runs in fp16
with a hi/lo mantissa split (2x11 bits ~ fp32-class precision) and per-row
balanced scaling s_k = sqrt(max|P_k|/max|x_k|) (the scales cancel inside each
product).  The K=128 stationary packs [X-hi(32) | X^2-hi(32) | biasA | biasB |
X-lo rows(62)] so ONE weight load + two matmuls per 128-sample chunk produce
both logits (hi@Phi + lo@Phi + hi@Plo accumulated in PSUM) and rule_out.
The -const bias is split 2^15*constA + constB across the two bias rows; the 2
X-rows with the smallest balanced magnitude lose their lo rows (neg. error).
Verified numerically: rel err ~4e-3 vs fp32 reference (gate 2e-2).

Sharding: X split along N across 8 cores; params replicated.  Host pre-
transposes/stacks X (no on-device transposes) and permutes samples so
partition p owns contiguous output rows [p*64,(p+1)*64) -> output DMAs are
per-partition contiguous.  Downstream per chunk: batched exp on ACT (PSUM
supertiles of 4 chunks), fused multiply+reduce work on DVE, normalized split
DVE/ACT, predictions = row-sum(strengths*rule)/(row-sum(strengths)+1e-8).
"""

import os
import sys

import numpy as np

for _p in ("/opt/trn_rl_repo",):
    if _p not in sys.path and os.path.isdir(_p):
        sys.path.insert(0, _p)

N, D, R = 65536, 32, 128
NCORES = 8
NSHARD = N // NCORES          # 8192 samples per core
KH = 2 * D + 2                # 66 hi rows: X, X^2, biasA, biasB
NLO = 128 - KH                # 62 lo rows
CHUNK = 128                   # samples per matmul (M dim)
NCHUNK = NSHARD // CHUNK      # 64
GROUP = 8                     # chunks per output-DMA group
NGROUP = NCHUNK // GROUP      # 8
SUB = 4                       # chunks per PSUM supertile
SCB = float(2.0**15)          # bias row A scale

_CACHE = {}

LAST_EXEC_NS = None
LAST_RESULTS = None


def _build_bass():
    import concourse.bacc as bacc
    import concourse.tile as tile
    from concourse import mybir

    f32 = mybir.dt.float32
    f16 = mybir.dt.float16
    nc = bacc.Bacc()

    xc = nc.declare_dram_parameter("xc", [128, NSHARD], f16, isOutput=False)
    pp1 = nc.declare_dram_parameter("pp1", [128, 2 * R], f16, isOutput=False)
    pp2 = nc.declare_dram_parameter("pp2", [128, R], f16, isOutput=False)
    o_str = nc.declare_dram_parameter("o_str", [NSHARD, R], f32, isOutput=True)
    o_norm = nc.declare_dram_parameter("o_norm", [NSHARD, R], f32, isOutput=True)
    o_pred = nc.declare_dram_parameter("o_pred", [NSHARD], f32, isOutput=True)

    with tile.TileContext(nc) as tc:
        with (
            tc.tile_pool(name="singles", bufs=1) as singles,
            tc.tile_pool(name="groups", bufs=NGROUP) as groups,
            tc.tile_pool(name="small", bufs=4) as small,
            tc.tile_pool(name="scratch", bufs=4) as scratch,
            tc.tile_pool(name="psum", bufs=3, space="PSUM") as psum_pool,
            tc.tile_pool(name="psum_d", bufs=1, space="PSUM") as psum_d_pool,
        ):
            # ---- load inputs -------------------------------------------------
            pp1_sb = singles.tile([128, 2 * R], f16)
            nc.sync.dma_start(out=pp1_sb[:, :], in_=pp1[:, :])
            pp2_sb = singles.tile([128, R], f16)
            nc.sync.dma_start(out=pp2_sb[:, :], in_=pp2[:, :])
            NPIECE = NGROUP  # one piece per group of chunks
            piece = NSHARD // NPIECE
            PREFETCH = 3     # pieces loaded up-front; the rest stream in
            xc_pieces = []
            for i in range(NPIECE):
                xp = singles.tile([128, piece], f16, tag=f"xc{i}", name=f"xcp{i}")
                xc_pieces.append(xp)

            def load_piece(i):
                # alternate issuing engine to parallelize descriptor gen
                eng = nc.sync if i % 2 == 0 else nc.scalar
                eng.dma_start(
                    out=xc_pieces[i][:, :], in_=xc[:, i * piece : (i + 1) * piece]
                )

            for i in range(PREFETCH):
                load_piece(i)

            pred_final = singles.tile([128, NCHUNK], f32)
            # permuted outputs: DRAM row n = p*NCHUNK + c
            o_str_v = o_str[:, :].rearrange("(p c) r -> p c r", p=CHUNK)
            o_norm_v = o_norm[:, :].rearrange("(p c) r -> p c r", p=CHUNK)

            # 1x1 "absorber" matmuls: walrus allows one sync-wait per
            # instruction; Bacc splits extras into costly event semaphores.
            # These soak up cross-engine waits so real matmuls carry <=1.
            psum_d = psum_d_pool.tile([1, 1], f32)

            def pe_sync(src_ap):
                nc.tensor.matmul(
                    psum_d[:, :], lhsT=src_ap, rhs=src_ap, start=True, stop=True
                )

            prev_str = None
            prev_recip = None

            for g in range(NGROUP):
                str_sb = groups.tile([128, GROUP, R], f32, tag="str")
                norm_sb = groups.tile([128, GROUP, R], f32, tag="norm")
                sums_g = small.tile([128, GROUP], f32, tag="sums")
                recip_g = small.tile([128, GROUP], f32, tag="recip")
                pred_raw = small.tile([128, GROUP], f32, tag="praw")

                # stream the next input piece while this group computes
                if g + PREFETCH < NPIECE:
                    load_piece(g + PREFETCH)

                # Absorb this group's cross-engine waits on PE: the piece DMA,
                # plus ACT/DVE progress (for PSUM supertile recycling).
                pe_sync(xc_pieces[g][0:1, 0:1])
                if prev_str is not None:
                    pe_sync(prev_str)
                if prev_recip is not None:
                    pe_sync(prev_recip)

                for q in range(GROUP // SUB):
                    # supertile: SUB chunks x [logits | rule], half-bank each
                    ps = psum_pool.tile([128, SUB, 2 * R], f32, tag="ps")
                    for j4 in range(SUB):
                        j = q * SUB + j4
                        lh = xc_pieces[g][:, j * CHUNK : (j + 1) * CHUNK]
                        # [ (hi@Phi + lo@Phi)  |  rule ]   (N=256)
                        nc.tensor.matmul(
                            ps[:, j4, :], lhsT=lh, rhs=pp1_sb[:, :],
                            start=True, stop=False,
                        )
                        # += hi@Plo (+ lo@Plo), logits half only
                        nc.tensor.matmul(
                            ps[:, j4, 0:R], lhsT=lh, rhs=pp2_sb[:, :],
                            start=False, stop=True, skip_group_check=True,
                        )
                    # strengths = exp(logits) for the whole supertile
                    nc.scalar.activation(
                        out=str_sb[:, q * SUB : (q + 1) * SUB, :],
                        in_=ps[:, :, 0:R],
                        func=mybir.ActivationFunctionType.Exp,
                    )
                    # tiny DVE read of the supertile absorbs the PE wait, so
                    # the tensor_mul below waits only on ACT
                    sacr = scratch.tile([1, 1], f32, tag="sacr")
                    nc.vector.tensor_copy(sacr, ps[0:1, 0, 0:1])
                    # t = strengths * rule_out; bf16 scratch so the row-sum
                    # reduce runs in the DVE's 2x packed mode
                    t_scr = scratch.tile([128, SUB, R], mybir.dt.bfloat16, tag=f"t{q}")
                    nc.vector.tensor_mul(
                        t_scr,
                        str_sb[:, q * SUB : (q + 1) * SUB, :],
                        ps[:, :, R : 2 * R],
                    )
                    nc.vector.reduce_sum(
                        out=pred_raw[:, q * SUB : (q + 1) * SUB],
                        in_=t_scr,
                        axis=mybir.AxisListType.X,
                    )
                # row-sums of strengths for the whole group
                nc.vector.reduce_sum(
                    out=sums_g, in_=str_sb, axis=mybir.AxisListType.X
                )
                # recip = 1 / (sums + 1e-8)
                nc.vector.tensor_scalar_add(out=recip_g, in0=sums_g, scalar1=1e-8)
                nc.vector.reciprocal(out=recip_g, in_=recip_g)
                # predictions for the group's chunks
                nc.vector.tensor_mul(
                    pred_final[:, g * GROUP : (g + 1) * GROUP], pred_raw, recip_g
                )
                # normalized = strengths * recip, first half DVE, second ACT
                H = GROUP // 2
                for j in range(GROUP):
                    if j < H:
                        nc.vector.tensor_scalar_mul(
                            out=norm_sb[:, j, :],
                            in0=str_sb[:, j, :],
                            scalar1=recip_g[:, j : j + 1],
                        )
                    else:
                        nc.scalar.activation(
                            out=norm_sb[:, j, :],
                            in_=str_sb[:, j, :],
                            func=mybir.ActivationFunctionType.Copy,
                            scale=recip_g[:, j : j + 1],
                        )

                # ---- group outputs (per-partition contiguous 4KB blocks) ----
                cs = slice(g * GROUP, (g + 1) * GROUP)
                csa = slice(g * GROUP, g * GROUP + H)
                csb = slice(g * GROUP + H, (g + 1) * GROUP)
                nc.sync.dma_start(out=o_str_v[:, cs, :], in_=str_sb)
                nc.sync.dma_start(out=o_norm_v[:, csa, :], in_=norm_sb[:, 0:H, :])
                nc.sync.dma_start(out=o_norm_v[:, csb, :], in_=norm_sb[:, H:GROUP, :])
                prev_str = str_sb[0:1, 0, 0:1]
                prev_recip = recip_g[0:1, 0:1]

            nc.sync.dma_start(
                out=o_pred[:].rearrange("(p c) -> p c", p=CHUNK), in_=pred_final[:, :]
            )

    nc.compile()
    return nc


def _host_prep(X, a, b, c):
    X = np.ascontiguousarray(X, dtype=np.float32)
    a = np.asarray(a, dtype=np.float32)
    b = np.asarray(b, dtype=np.float32)
    c = np.asarray(c, dtype=np.float32)
    f16 = np.float16

    b_cl = np.maximum(b, 1e-8)
    w = 1.0 / (2.0 * b_cl * b_cl)            # [R, D]
    const = (a * a * w).sum(axis=1)          # [R]

    # hi rows 0..63 = [X | X^2], 64 biasA (x-side 2^15), 65 biasB (x-side 1)
    constA16 = (-const / SCB).astype(f16).astype(np.float32)
    constB = (-const - SCB * constA16).astype(np.float32)
    constB16 = constB.astype(f16).astype(np.float32)
    constB_lo = constB - constB16

    PLd = np.zeros((2 * D, R), np.float32)   # data rows of PL
    PLd[0:D] = (2.0 * a * w).T
    PLd[D : 2 * D] = -w.T

    xcol_max = np.empty(2 * D, np.float32)
    xcol_max[0:D] = np.abs(X).max(axis=0)
    xcol_max[D : 2 * D] = (X * X).max(axis=0)
    pmax = np.abs(PLd).max(axis=1)
    s = np.sqrt(np.maximum(pmax, 1e-12) / np.maximum(xcol_max, 1e-12)).astype(
        np.float32
    )

    Psc = PLd / s[:, None]
    Phi = Psc.astype(f16)
    Plo = (Psc - Phi.astype(np.float32)).astype(f16)

    # lo rows: drop the 2 data rows with smallest balanced magnitude
    prod = (xcol_max * s) * pmax / s
    order = np.argsort(prod)
    sel = np.sort(order[2:])                 # 62 kept rows (indices into 0..63)
    assert len(sel) == NLO

    rule_hi = np.zeros((2 * D, R), np.float32)
    rule_hi[0:D] = c[:, :D].T / s[0:D, None]
    rule16 = rule_hi.astype(f16)

    pp1 = np.zeros((128, 2 * R), f16)
    pp1[0 : 2 * D, 0:R] = Phi
    pp1[2 * D, 0:R] = constA16.astype(f16)
    pp1[2 * D + 1, 0:R] = constB16.astype(f16)
    pp1[KH:128, 0:R] = Phi[sel]
    pp1[0 : 2 * D, R : 2 * R] = rule16
    pp1[2 * D + 1, R : 2 * R] = c[:, D].astype(f16)
    pp1[KH:128, R : 2 * R] = rule16[sel]

    pp2 = np.zeros((128, R), f16)
    pp2[0 : 2 * D] = Plo
    pp2[2 * D + 1] = constB_lo.astype(f16)
    pp2[KH:128] = Plo[sel]  # Xlo @ Plo (lo*lo term, free accuracy)

    xcs = []
    for i in range(NCORES):
        Xs = X[i * NSHARD : (i + 1) * NSHARD]
        # permute: device chunk c / partition p = original sample p*64+c
        Xp = Xs.reshape(CHUNK, NCHUNK, D).transpose(1, 0, 2).reshape(NSHARD, D)
        xs = np.empty((2 * D, NSHARD), np.float32)
        xs[0:D] = Xp.T * s[0:D, None]
        xs[D : 2 * D] = (Xp * Xp).T * s[D : 2 * D, None]
        xh = xs.astype(f16)
        xl = (xs - xh.astype(np.float32)).astype(f16)
        xcq = np.empty((128, NSHARD), f16)
        xcq[0 : 2 * D] = xh
        xcq[2 * D] = np.float16(SCB)
        xcq[2 * D + 1] = np.float16(1.0)
        xcq[KH:128] = xl[sel]
        xcs.append(np.ascontiguousarray(xcq))
    return xcs, pp1, pp2


def _install_ntff_hook():
    """The agent image's antenv lacks axon_hooks; synthesize it so
    run_bass_kernel_spmd(trace=True) can capture NTFF profiles."""
    import types

    if "antenv.axon_hooks" in sys.modules:
        return
    try:
        sys.path.insert(0, "/root/.axon_site")
        from trn_agent_boot.trn_boot import _ntff_profile_via_ctypes

        hook = _ntff_profile_via_ctypes("/opt/axon/libaxon_pjrt.so")
    except Exception:
        return
    mod = types.ModuleType("antenv.axon_hooks")
    holder = {"h": hook}
    mod.set_axon_ntff_profile_hook = lambda h: holder.__setitem__("h", h)
    mod.get_axon_ntff_profile_hook = lambda: holder.get("h")
    sys.modules["antenv.axon_hooks"] = mod
    import antenv

    antenv.axon_hooks = mod


def kernel(X, a, b, c):
    global LAST_EXEC_NS, LAST_RESULTS
    from concourse.bass_utils import run_bass_kernel_spmd

    if "nc" not in _CACHE:
        _CACHE["nc"] = _build_bass()
    nc = _CACHE["nc"]

    xcs, pp1, pp2 = _host_prep(X, a, b, c)
    in_maps = [{"xc": xcs[i], "pp1": pp1, "pp2": pp2} for i in range(NCORES)]

    trace = os.environ.get("KERNEL_TRACE", "0") == "1"
    if trace:
        _install_ntff_hook()
    res = run_bass_kernel_spmd(nc, in_maps, core_ids=list(range(NCORES)), trace=trace)
    LAST_EXEC_NS = res.exec_time_ns
    LAST_RESULTS = res

    preds = np.concatenate([res.results[i]["o_pred"] for i in range(NCORES)], axis=0)
    strs = np.concatenate([res.results[i]["o_str"] for i in range(NCORES)], axis=0)
    norms = np.concatenate([res.results[i]["o_norm"] for i in range(NCORES)], axis=0)
    return (preds, strs, norms)
